# revision 19
# baseline (speedup 1.0000x reference)
"""DeepseekV3 MLA attention kernel for 8 Trainium2 NeuronCores.

Sharding: core c handles batch b = c // 4 and query rows
[ (c%4)*QB, (c%4+1)*QB ) for ALL heads.  K/V are computed for the full
sequence on every core (duplicated across the 4 cores of a batch), the
o-projection is fully local, so no collectives are needed.

Everything on-device runs in a feature-major ("transposed") layout so
that matmul contractions always have the contracted dim on partitions:
  hsT       [HID, S]     (host-transposed hidden states)
  q_aT      [QL,  QB]    (fp32 accum; rms-normed into a bf16 copy)
  qT_h      [QHD, QB]
  ckvT      [KVL+ROPE, S]
  k_nopeT_h [NOPE, S]
  V_h       [S, VD]      (row-major, for attn@V stationary operand)
  scoresT   [S_k, QB]    (fp32; softmax stats across partitions via
                          tree-max + PE transposes; probs cast to bf16)

Heavy matmuls run in bf16 (the fp32/f32r weight-load path tolerates at
most one embedded sync wait in walrus codegen, which Tile's scheduler
cannot guarantee); all accumulation, norms and softmax statistics stay
fp32.  Broadcast of per-query stats across partitions is done with DMA
partition-replication (exact, no PE involvement).

Host-side weight preprocessing (exact, zero device cost):
  - RMS-norm gammas folded into the following projection's input dim
  - softmax scale folded into q_b weights
  - RoPE interleave permutation folded into q_b / kv_a output rows
"""

import sys

import ml_dtypes
import numpy as np

for _p in ("/opt/trn_rl_repo",):
    if _p not in sys.path:
        sys.path.insert(0, _p)

# ---- problem dims (hardcoded per spec) ----
B, S, HID = 2, 2048, 2048
H = 16
NOPE, ROPE, VD = 128, 64, 128
QHD = NOPE + ROPE            # 192
QL, KVL = 1536, 512
BASE = 10000.0
EPS = 1e-6
SCALE = QHD ** -0.5
NCORES = 8
CPB = NCORES // B            # cores per batch = 4
QB = S // CPB                # query rows per core = 512

MM_DT_NAME = "bfloat16"      # heavy-matmul operand dtype


def _cfg(S=S, HID=HID, H=H, QL=QL, KVL=KVL, B=B, NCORES=NCORES):
    """Derived loop bounds; parameterized so tests can shrink dims."""
    cpb = NCORES // B
    qb = S // cpb
    assert qb <= 512
    return dict(
        S=S, HID=HID, H=H, QL=QL, KVL=KVL, B=B, NCORES=NCORES,
        CPB=cpb, QB=qb,
        HC=HID // 128,     # hidden k-chunks
        QLC=QL // 128,     # q low-rank chunks
        KVC=KVL // 128,    # kv low-rank chunks
        SC=S // 128,       # sequence chunks (keys)
        ST=S // 512,       # sequence 512-tiles
        NT=HID // 512,     # output col tiles
        RC=qb // 128,      # query row chunks
        VB=2,              # heads per V block
    )


def build_program(cfg=None, mm_dt_name=MM_DT_NAME, split_waits=True):
    import concourse.bass as bass
    import concourse.tile as tile
    from concourse import mybir
    from concourse.masks import make_identity

    if cfg is None:
        cfg = _cfg()
    S_, HID_, H_, QL_, KVL_ = cfg["S"], cfg["HID"], cfg["H"], cfg["QL"], cfg["KVL"]
    QB_, HC, QLC, KVC, SC, ST, NT, RC, VB = (
        cfg["QB"], cfg["HC"], cfg["QLC"], cfg["KVC"], cfg["SC"], cfg["ST"],
        cfg["NT"], cfg["RC"], cfg["VB"])

    f32 = mybir.dt.float32
    mdt = getattr(mybir.dt, mm_dt_name)
    Alu = mybir.AluOpType
    Act = mybir.ActivationFunctionType
    Ax = mybir.AxisListType

    nc = bass.Bass()
    mtm = nc.tensor.matmul

    # ---- I/O ----
    hsT = nc.dram_tensor("hsT", [HID_, S_], mdt, kind="ExternalInput")
    hsqT = nc.dram_tensor("hsqT", [HID_, QB_], mdt, kind="ExternalInput")
    qawT = nc.dram_tensor("qawT", [HID_, QL_], mdt, kind="ExternalInput")
    qbwT = nc.dram_tensor("qbwT", [QL_, H_ * QHD], mdt, kind="ExternalInput")
    kvawT = nc.dram_tensor("kvawT", [HID_, KVL_ + ROPE], mdt, kind="ExternalInput")
    kvbkT = nc.dram_tensor("kvbkT", [KVL_, H_ * NOPE], mdt, kind="ExternalInput")
    kvbvT = nc.dram_tensor("kvbvT", [KVL_, H_ * VD], mdt, kind="ExternalInput")
    owT = nc.dram_tensor("owT", [H_ * VD, HID_], mdt, kind="ExternalInput")
    maskT = nc.dram_tensor("maskT", [S_, QB_], mdt, kind="ExternalInput")
    cosT = nc.dram_tensor("cosT", [ROPE // 2, S_], f32, kind="ExternalInput")
    sinT = nc.dram_tensor("sinT", [ROPE // 2, S_], f32, kind="ExternalInput")
    cosqT = nc.dram_tensor("cosqT", [ROPE // 2, QB_], f32, kind="ExternalInput")
    sinqT = nc.dram_tensor("sinqT", [ROPE // 2, QB_], f32, kind="ExternalInput")
    out = nc.dram_tensor("out", [QB_, HID_], f32, kind="ExternalOutput")
    attn_scr = nc.dram_tensor("attn_scr", [H_, VD, QB_], mdt)

    R2 = ROPE // 2

    with tile.TileContext(nc) as tc:
        with (
            tc.tile_pool(name="poolA", bufs=1) as pA,
            tc.tile_pool(name="psA", bufs=2, space="PSUM") as psA,
        ):
            # ---- constants ----
            ident = pA.tile([128, 128], f32)
            make_identity(nc, ident)
            ones_colr = pA.tile([128, 1], mdt)
            nc.vector.memset(ones_colr, 1.0)
            ones_col = pA.tile([128, 1], f32)
            nc.vector.memset(ones_col, 1.0)
            ones_row = pA.tile([1, 128], f32)
            nc.vector.memset(ones_row, 1.0)
            zero_col = pA.tile([128, 1], f32)
            nc.vector.memset(zero_col, 0.0)
            eps_col = pA.tile([128, 1], f32)
            nc.vector.memset(eps_col, EPS)
            cos_q = pA.tile([R2, QB_], f32)
            sin_q = pA.tile([R2, QB_], f32)
            nc.sync.dma_start(out=cos_q, in_=cosqT[:, :])
            nc.sync.dma_start(out=sin_q, in_=sinqT[:, :])

            with tc.tile_pool(name="poolB", bufs=1) as pB:
                qa_bf = pB.tile([128, QLC, QB_], mdt)     # normed q_aT
                ckv_bf = pB.tile([128, KVC, S_], mdt)     # normed ckvT
                kpe_rope = pB.tile([ROPE, S_], mdt)       # rope'd shared k_pe

                # ================= P1: a-projections + norms =================
                with (
                    tc.tile_pool(name="p1acc", bufs=1) as p1acc,
                    tc.tile_pool(name="p1", bufs=5) as p1,
                    tc.tile_pool(name="p1b", bufs=2) as p1b,
                    tc.tile_pool(name="ps_var", bufs=2, space="PSUM") as ps_var,
                    tc.tile_pool(name="ps_vb", bufs=1, space="PSUM") as ps_vb,
                ):
                    qa_acc = p1acc.tile([128, QLC, QB_], f32)
                    ckv_acc = p1acc.tile([128, KVC, S_], f32)
                    kpe_acc = p1acc.tile([ROPE, S_], f32)
                    cos_k = p1b.tile([R2, S_], f32, tag="cosk", bufs=1)
                    sin_k = p1b.tile([R2, S_], f32, tag="sink", bufs=1)
                    nc.sync.dma_start(out=cos_k, in_=cosT[:, :])
                    nc.sync.dma_start(out=sin_k, in_=sinT[:, :])

                    for g in range(0, HC, 4):
                        hs_ch, hsq_ch, qaw_ch, kvaw_ch = [], [], [], []
                        for i in range(4):
                            kc = g + i
                            hs_t = p1.tile([128, S_], mdt, tag="hs")
                            nc.sync.dma_start(out=hs_t, in_=hsT[kc * 128:(kc + 1) * 128, :])
                            hs_ch.append(hs_t)
                            hsq_t = p1.tile([128, QB_], mdt, tag="hsq")
                            nc.sync.dma_start(out=hsq_t, in_=hsqT[kc * 128:(kc + 1) * 128, :])
                            hsq_ch.append(hsq_t)
                            qaw_t = p1.tile([128, QL_], mdt, tag="qaw", bufs=4)
                            nc.sync.dma_start(out=qaw_t, in_=qawT[kc * 128:(kc + 1) * 128, :])
                            qaw_ch.append(qaw_t)
                            kvaw_t = p1.tile([128, KVL_ + ROPE], mdt, tag="kvaw", bufs=4)
                            nc.sync.dma_start(out=kvaw_t, in_=kvawT[kc * 128:(kc + 1) * 128, :])
                            kvaw_ch.append(kvaw_t)

                        # q_aT chunks [128, QB]
                        for mc in range(QLC):
                            ps_q = psA.tile([128, QB_], f32, tag="ps")
                            for i in range(4):
                                mtm(ps_q, qaw_ch[i][:, mc * 128:(mc + 1) * 128],
                                    hsq_ch[i], start=(i == 0), stop=(i == 3))
                            dst = qa_acc[:, mc, :]
                            if g == 0:
                                nc.scalar.copy(dst, ps_q)
                            else:
                                nc.vector.tensor_tensor(out=dst, in0=ps_q, in1=dst, op=Alu.add)

                        # ckvT chunks [128, S] (+ rope chunk [64, S])
                        for mc in range(KVC + 1):
                            pe_part = (mc == KVC)
                            mrows = ROPE if pe_part else 128
                            for nt in range(ST):
                                ps_kv = psA.tile([128, 512], f32, tag="ps")
                                for i in range(4):
                                    mtm(ps_kv[:mrows, :],
                                        kvaw_ch[i][:, mc * 128:mc * 128 + mrows],
                                        hs_ch[i][:, nt * 512:(nt + 1) * 512],
                                        start=(i == 0), stop=(i == 3))
                                dst = (kpe_acc[:, nt * 512:(nt + 1) * 512] if pe_part
                                       else ckv_acc[:, mc, nt * 512:(nt + 1) * 512])
                                if g == 0:
                                    nc.scalar.copy(dst, ps_kv[:mrows, :])
                                else:
                                    nc.vector.tensor_tensor(out=dst, in0=ps_kv[:mrows, :],
                                                            in1=dst, op=Alu.add)

                    # ---- RMS norm of q_aT (partition sum via ones-matmul) ----
                    ps_v = ps_var.tile([1, QB_], f32, tag="v")
                    for mc in range(QLC):
                        sq = p1b.tile([128, QB_], f32, tag="sq")
                        nc.scalar.activation(sq, qa_acc[:, mc, :], Act.Square, bias=zero_col)
                        mtm(ps_v, ones_col, sq, start=(mc == 0), stop=(mc == QLC - 1))
                    rs_tmp = p1b.tile([1, QB_], f32, tag="rs", bufs=1)
                    nc.scalar.activation(rs_tmp, ps_v, Act.Sqrt, bias=eps_col[:1],
                                         scale=1.0 / QL_)
                    rs_q = p1b.tile([1, QB_], f32, tag="rsq", bufs=1)
                    nc.vector.reciprocal(rs_q, rs_tmp)
                    rsq_b = ps_vb.tile([128, QB_], f32, tag="vb")
                    mtm(rsq_b, ones_row, rs_q)
                    for mc in range(QLC):
                        nc.vector.tensor_tensor(out=qa_bf[:, mc, :], in0=qa_acc[:, mc, :],
                                                in1=rsq_b, op=Alu.mult)

                    # ---- RMS norm of ckvT ----
                    for nt in range(ST):
                        ps_vk = ps_var.tile([1, 512], f32, tag="v")
                        for mc in range(KVC):
                            sqk = p1b.tile([128, 512], f32, tag="sq")
                            nc.scalar.activation(sqk, ckv_acc[:, mc, nt * 512:(nt + 1) * 512],
                                                 Act.Square, bias=zero_col)
                            mtm(ps_vk, ones_col, sqk, start=(mc == 0), stop=(mc == KVC - 1))
                        rs_tmpk = p1b.tile([1, 512], f32, tag="rs", bufs=1)
                        nc.scalar.activation(rs_tmpk, ps_vk, Act.Sqrt, bias=eps_col[:1],
                                             scale=1.0 / KVL_)
                        rs_kv = p1b.tile([1, 512], f32, tag="rsq", bufs=1)
                        nc.vector.reciprocal(rs_kv, rs_tmpk)
                        rskv_b = ps_vb.tile([128, 512], f32, tag="vb")
                        mtm(rskv_b, ones_row, rs_kv)
                        for mc in range(KVC):
                            nc.vector.tensor_tensor(
                                out=ckv_bf[:, mc, nt * 512:(nt + 1) * 512],
                                in0=ckv_acc[:, mc, nt * 512:(nt + 1) * 512],
                                in1=rskv_b, op=Alu.mult)

                    # ---- RoPE on shared k_pe [ROPE, S], 512-col tiles ----
                    # engines need matching start partitions: bring the hi
                    # half down to partition 0 via DMA, compute there, then
                    # DMA-assemble the 64-row result.
                    for nt in range(ST):
                        sl = slice(nt * 512, (nt + 1) * 512)
                        kpe_hi = p1b.tile([R2, 512], f32, tag="kpehi", bufs=1)
                        nc.sync.dma_start(out=kpe_hi, in_=kpe_acc[R2:, sl])
                        t0 = p1b.tile([R2, 512], f32, tag="t0", bufs=1)
                        t1 = p1b.tile([R2, 512], f32, tag="t1", bufs=1)
                        y_lo = p1b.tile([R2, 512], mdt, tag="ylo", bufs=1)
                        y_hi = p1b.tile([R2, 512], mdt, tag="yhi", bufs=1)
                        nc.vector.tensor_tensor(out=t0, in0=kpe_acc[:R2, sl], in1=cos_k[:, sl], op=Alu.mult)
                        nc.vector.tensor_tensor(out=t1, in0=kpe_hi, in1=sin_k[:, sl], op=Alu.mult)
                        nc.vector.tensor_tensor(out=y_lo, in0=t0, in1=t1, op=Alu.subtract)
                        nc.vector.tensor_tensor(out=t0, in0=kpe_hi, in1=cos_k[:, sl], op=Alu.mult)
                        nc.vector.tensor_tensor(out=t1, in0=kpe_acc[:R2, sl], in1=sin_k[:, sl], op=Alu.mult)
                        nc.vector.tensor_tensor(out=y_hi, in0=t0, in1=t1, op=Alu.add)
                        nc.sync.dma_start(out=kpe_rope[:R2, sl], in_=y_lo)
                        nc.sync.dma_start(out=kpe_rope[R2:, sl], in_=y_hi)

                # ================= P2: per-head attention =================
                with (
                    tc.tile_pool(name="p2", bufs=1) as p2,
                    tc.tile_pool(name="p2s", bufs=3) as p2s,
                    tc.tile_pool(name="p2d", bufs=2) as p2d,
                    tc.tile_pool(name="ps_at", bufs=2, space="PSUM") as ps_at,
                    tc.tile_pool(name="ps_m1", bufs=1, space="PSUM") as ps_m1,
                    tc.tile_pool(name="ps_qr", bufs=1, space="PSUM") as ps_qrp,
                    tc.tile_pool(name="ps_mb", bufs=1, space="PSUM") as ps_mb,
                    tc.tile_pool(name="ps_ib", bufs=1, space="PSUM") as ps_ib,
                ):
                    mask_sb = p2.tile([128, SC, QB_], mdt)
                    for kt in range(SC):
                        nc.sync.dma_start(out=mask_sb[:, kt, :],
                                          in_=maskT[kt * 128:(kt + 1) * 128, :])

                    for h in range(H_):
                        hb = h % VB
                        # ---- V block (row-major) for VB heads ----
                        if hb == 0:
                            v_blk = p2.tile([128, SC, VB * VD], mdt, tag="vblk")
                            kvbv_ch = []
                            for cc in range(KVC):
                                kvbv_t = p2s.tile([128, VB * VD], mdt, tag="kvbv",
                                                  bufs=KVC + 1)
                                nc.sync.dma_start(
                                    out=kvbv_t,
                                    in_=kvbvT[cc * 128:(cc + 1) * 128,
                                              h * VD:(h + VB) * VD])
                                kvbv_ch.append(kvbv_t)
                            for st in range(SC):
                                ps_vv = psA.tile([128, VB * VD], f32, tag="ps")
                                for cc in range(KVC):
                                    mtm(ps_vv, ckv_bf[:, cc, st * 128:(st + 1) * 128],
                                        kvbv_ch[cc], start=(cc == 0), stop=(cc == KVC - 1))
                                nc.scalar.copy(v_blk[:, st, :], ps_vv)

                        # ---- q_bT for head h: qT [QHD, QB] ----
                        qbw_ch = []
                        for kc in range(QLC):
                            qbw_t = p2s.tile([128, QHD], mdt, tag="qbw", bufs=QLC + 1)
                            nc.sync.dma_start(out=qbw_t,
                                              in_=qbwT[kc * 128:(kc + 1) * 128,
                                                       h * QHD:(h + 1) * QHD])
                            qbw_ch.append(qbw_t)
                        ps_qn = psA.tile([128, QB_], f32, tag="ps")
                        ps_qr = ps_qrp.tile([ROPE, QB_], f32, tag="qr")
                        for kc in range(QLC):
                            mtm(ps_qn, qbw_ch[kc][:, :NOPE], qa_bf[:, kc, :],
                                start=(kc == 0), stop=(kc == QLC - 1))
                        for kc in range(QLC):
                            mtm(ps_qr, qbw_ch[kc][:, NOPE:], qa_bf[:, kc, :],
                                start=(kc == 0), stop=(kc == QLC - 1))
                        qt_n = p2d.tile([128, QB_], mdt, tag="qtn")
                        nc.scalar.copy(qt_n, ps_qn)
                        # RoPE on q_pe (psum upper half -> partition 0 first)
                        qt_r = p2d.tile([ROPE, QB_], mdt, tag="qtr")
                        q_hi = p2d.tile([R2, QB_], f32, tag="qhi", bufs=2)
                        nc.scalar.copy(q_hi, ps_qr[R2:, :])
                        tq0 = p2d.tile([R2, QB_], f32, tag="tq0", bufs=1)
                        tq1 = p2d.tile([R2, QB_], f32, tag="tq1", bufs=1)
                        qy_lo = p2d.tile([R2, QB_], mdt, tag="qylo", bufs=2)
                        qy_hi = p2d.tile([R2, QB_], mdt, tag="qyhi", bufs=2)
                        nc.vector.tensor_tensor(out=tq0, in0=ps_qr[:R2, :], in1=cos_q, op=Alu.mult)
                        nc.vector.tensor_tensor(out=tq1, in0=q_hi, in1=sin_q, op=Alu.mult)
                        nc.vector.tensor_tensor(out=qy_lo, in0=tq0, in1=tq1, op=Alu.subtract)
                        nc.vector.tensor_tensor(out=tq0, in0=q_hi, in1=cos_q, op=Alu.mult)
                        nc.vector.tensor_tensor(out=tq1, in0=ps_qr[:R2, :], in1=sin_q, op=Alu.mult)
                        nc.vector.tensor_tensor(out=qy_hi, in0=tq0, in1=tq1, op=Alu.add)
                        nc.sync.dma_start(out=qt_r[:R2, :], in_=qy_lo)
                        nc.sync.dma_start(out=qt_r[R2:, :], in_=qy_hi)

                        # ---- K_nopeT for head h [NOPE, S] ----
                        kvbk_ch = []
                        for cc in range(KVC):
                            kvbk_t = p2s.tile([128, NOPE], mdt, tag="kvbk", bufs=KVC + 1)
                            nc.sync.dma_start(out=kvbk_t,
                                              in_=kvbkT[cc * 128:(cc + 1) * 128,
                                                        h * NOPE:(h + 1) * NOPE])
                            kvbk_ch.append(kvbk_t)
                        k_sb = p2.tile([128, S_], mdt, tag="ksb")
                        for st in range(ST):
                            ps_k = psA.tile([128, 512], f32, tag="ps")
                            for cc in range(KVC):
                                mtm(ps_k, kvbk_ch[cc], ckv_bf[:, cc, st * 512:(st + 1) * 512],
                                    start=(cc == 0), stop=(cc == KVC - 1))
                            nc.scalar.copy(k_sb[:, st * 512:(st + 1) * 512], ps_k)

                        # ---- scoresT [S_k, QB] + mask (fp32) ----
                        sc_t = p2.tile([128, SC, QB_], f32, tag="sc")
                        for kt in range(SC):
                            ps_s = psA.tile([128, QB_], f32, tag="ps")
                            mtm(ps_s, k_sb[:, kt * 128:(kt + 1) * 128], qt_n,
                                start=True, stop=False)
                            mtm(ps_s, kpe_rope[:, kt * 128:(kt + 1) * 128], qt_r,
                                start=False, stop=True)
                            nc.vector.tensor_tensor(out=sc_t[:, kt, :], in0=ps_s,
                                                    in1=mask_sb[:, kt, :], op=Alu.add)

                        # ---- per-query max (tree over k-chunks, then transpose) ----
                        tmax = p2d.tile([128, QB_], f32, tag="tmax")
                        nc.vector.tensor_copy(tmax, sc_t[:, 0, :])
                        for kt in range(1, SC):
                            nc.vector.tensor_tensor(out=tmax, in0=tmax, in1=sc_t[:, kt, :],
                                                    op=Alu.max)
                        maxrow = p2d.tile([1, QB_], f32, tag="maxrow", bufs=1)
                        for i in range(RC):
                            ps_t = ps_m1.tile([128, 128], f32, tag="m")
                            nc.tensor.transpose(ps_t, tmax[:, i * 128:(i + 1) * 128], ident)
                            mq = p2d.tile([128, 1], f32, tag="mq")
                            nc.vector.reduce_max(out=mq, in_=ps_t, axis=Ax.X)
                            ps_r = ps_m1.tile([1, 128], f32, tag="m")
                            nc.tensor.transpose(ps_r, mq, ident)
                            nc.vector.tensor_copy(maxrow[:, i * 128:(i + 1) * 128], ps_r)
                        mx_b = ps_mb.tile([128, QB_], f32, tag="mb")
                        mtm(mx_b, ones_row, maxrow)

                        # ---- probs = exp(scores - max), bf16 ----
                        pr_t = p2.tile([128, SC, QB_], mdt, tag="pr")
                        for kt in range(SC):
                            nc.vector.tensor_tensor(out=sc_t[:, kt, :], in0=sc_t[:, kt, :],
                                                    in1=mx_b, op=Alu.subtract)
                            nc.scalar.activation(pr_t[:, kt, :], sc_t[:, kt, :], Act.Exp,
                                                 bias=zero_col)

                        # ---- sum + attn @ V ----
                        ps_sm = ps_m1.tile([1, QB_], f32, tag="m")
                        for kt in range(SC):
                            mtm(ps_sm, ones_colr, pr_t[:, kt, :],
                                start=(kt == 0), stop=(kt == SC - 1))
                        ps_o = ps_at.tile([128, QB_], f32, tag="o")
                        for kt in range(SC):
                            mtm(ps_o, v_blk[:, kt, hb * VD:(hb + 1) * VD], pr_t[:, kt, :],
                                start=(kt == 0), stop=(kt == SC - 1))
                        inv_s = p2d.tile([1, QB_], f32, tag="invs", bufs=1)
                        nc.vector.reciprocal(inv_s, ps_sm)
                        ps_iv = ps_ib.tile([128, QB_], f32, tag="ib")
                        mtm(ps_iv, ones_row, inv_s)
                        iv_sb = p2d.tile([128, QB_], f32, tag="ivb", bufs=2)
                        nc.vector.tensor_copy(iv_sb, ps_iv)
                        ao_sb = p2d.tile([128, QB_], mdt, tag="aosb")
                        nc.vector.tensor_tensor(out=ao_sb, in0=ps_o, in1=iv_sb, op=Alu.mult)
                        nc.sync.dma_start(out=attn_scr[h], in_=ao_sb)

            # ================= P3: o-projection =================
            with (
                tc.tile_pool(name="p3", bufs=3) as p3,
                tc.tile_pool(name="p3o", bufs=2) as p3o,
                tc.tile_pool(name="ps_oo", bufs=4, space="PSUM") as ps_oo,
            ):
                for nt in range(NT):
                    ps_list = [ps_oo.tile([128, 512], f32, tag="oo", name=f"oo{nt}_{i}")
                               for i in range(RC)]
                    for h in range(H_):
                        owt_t = p3.tile([128, 512], mdt, tag="owt")
                        nc.sync.dma_start(out=owt_t,
                                          in_=owT[h * VD:(h + 1) * VD, nt * 512:(nt + 1) * 512])
                        at_t = p3.tile([128, QB_], mdt, tag="at")
                        nc.sync.dma_start(out=at_t, in_=attn_scr[h])
                        for rc in range(RC):
                            mtm(ps_list[rc], at_t[:, rc * 128:(rc + 1) * 128], owt_t,
                                start=(h == 0), stop=(h == H_ - 1))
                    for rc in range(RC):
                        o_sb = p3o.tile([128, 512], f32, tag="osb")
                        nc.scalar.copy(o_sb, ps_list[rc])
                        nc.sync.dma_start(out=out[rc * 128:(rc + 1) * 128,
                                                  nt * 512:(nt + 1) * 512],
                                          in_=o_sb)
    if split_waits:
        _split_excess_waits(nc)
    return nc



def _split_excess_waits(nc, max_w=1):
    """Walrus codegen allows very few embedded sync waits per instruction
    (1 for DMA descriptors and the matmul weight-load path).  Move excess
    waits into standalone EventSemaphore instructions on the same engine,
    inserted immediately before, preserving semantics."""
    import bass_rust
    from concourse import mybir

    k = 0
    for bb in nc.main_func.blocks:
        il = bb.instructions
        i = 0
        while i < len(il):
            ins = il[i]
            si = getattr(ins, "sync_info", None)
            if si is not None and len(si.on_wait) > max_w:
                waits = list(si.on_wait)
                extra, keep = waits[:-max_w], waits[-max_w:]
                for j in range(0, len(extra), max_w):
                    ev = mybir.InstEventSemaphore(name=f"wsplit{k}", engine=ins.engine)
                    k += 1
                    ev.sync_info = bass_rust.SyncInfo(
                        on_wait=extra[j:j + max_w], on_update=[])
                    il.insert(i, ev)
                    i += 1
                ins.sync_info = bass_rust.SyncInfo(
                    on_wait=keep, on_update=list(si.on_update))
            i += 1


# interleave permutation: new row j <- old row perm[j]
_PERM64 = np.concatenate([np.arange(0, ROPE, 2), np.arange(1, ROPE, 2)])


def host_prep(hidden_states, attention_mask, position_ids,
              q_a_w, q_a_ln_w, q_b_w, kv_a_w, kv_a_ln_w, kv_b_w, o_w):
    """Build the 8 per-core input maps."""
    f = np.float32
    bf = ml_dtypes.bfloat16

    def c(x, dt=bf):
        return np.ascontiguousarray(x.astype(dt))

    hidden_states = np.asarray(hidden_states, f)
    attention_mask = np.asarray(attention_mask, f)
    position_ids = np.asarray(position_ids)
    q_a_w = np.asarray(q_a_w, f); q_a_ln_w = np.asarray(q_a_ln_w, f)
    q_b_w = np.asarray(q_b_w, f); kv_a_w = np.asarray(kv_a_w, f)
    kv_a_ln_w = np.asarray(kv_a_ln_w, f); kv_b_w = np.asarray(kv_b_w, f)
    o_w = np.asarray(o_w, f)

    qawT = c(q_a_w.T)                                    # [HID, QL]
    qbw_eff = q_b_w * (q_a_ln_w[None, :] * SCALE)        # fold gamma + scale
    qbw_eff = qbw_eff.reshape(H, QHD, QL)
    qbw_eff[:, NOPE:, :] = qbw_eff[:, NOPE + _PERM64, :]  # rope interleave
    qbwT = c(qbw_eff.reshape(H * QHD, QL).T)             # [QL, H*QHD]

    kvaw_p = kv_a_w.copy()
    kvaw_p[KVL:] = kv_a_w[KVL + _PERM64]                 # rope interleave
    kvawT = c(kvaw_p.T)                                  # [HID, KVL+ROPE]

    kvb_eff = (kv_b_w * kv_a_ln_w[None, :]).reshape(H, NOPE + VD, KVL)
    kvbkT = c(kvb_eff[:, :NOPE, :].reshape(H * NOPE, KVL).T)   # [KVL, H*NOPE]
    kvbvT = c(kvb_eff[:, NOPE:, :].reshape(H * VD, KVL).T)     # [KVL, H*VD]
    owT = c(o_w.T)                                       # [H*VD, HID]

    inv_freq = (1.0 / (BASE ** (np.arange(0, ROPE, 2) / ROPE))).astype(np.float64)
    in_maps = []
    for core in range(NCORES):
        b, blk = divmod(core, CPB)
        r0 = blk * QB
        hsT = np.ascontiguousarray(hidden_states[b].T)   # [HID, S] fp32
        pos = position_ids[b].astype(np.float64)
        freqs = inv_freq[:, None] * pos[None, :]         # [R2, S]
        cosT = np.cos(freqs).astype(f)
        sinT = np.sin(freqs).astype(f)
        in_maps.append({
            "hsT": c(hsT),
            "hsqT": c(hsT[:, r0:r0 + QB]),
            "qawT": qawT, "qbwT": qbwT, "kvawT": kvawT,
            "kvbkT": kvbkT, "kvbvT": kvbvT, "owT": owT,
            "maskT": c(attention_mask[b, 0, r0:r0 + QB, :].T),
            "cosT": cosT, "sinT": sinT,
            "cosqT": np.ascontiguousarray(cosT[:, r0:r0 + QB]),
            "sinqT": np.ascontiguousarray(sinT[:, r0:r0 + QB]),
        })
    return in_maps


def assemble_output(results):
    out = np.empty((B, S, HID), np.float32)
    for core in range(NCORES):
        b, blk = divmod(core, CPB)
        r0 = blk * QB
        out[b, r0:r0 + QB, :] = results[core]["out"]
    return out


def kernel(hidden_states, attention_mask, position_ids,
           q_a_w, q_a_ln_w, q_b_w, kv_a_w, kv_a_ln_w, kv_b_w, o_w):
    from concourse.bass_utils import run_bass_kernel_spmd

    in_maps = host_prep(hidden_states, attention_mask, position_ids,
                        q_a_w, q_a_ln_w, q_b_w, kv_a_w, kv_a_ln_w, kv_b_w, o_w)
    nc = build_program()
    res = run_bass_kernel_spmd(nc, in_maps, list(range(NCORES)))
    return assemble_output(res.results)
